# revision 3
# baseline (speedup 1.0000x reference)
"""Exact attention (B=2, N=2048, H=16, D=64, fp32) on 8 Trainium2 NeuronCores.

v3 design, built from HW microbenchmarks (mb.py):
  - ACT exp measures ~1.23 ns/elem (1.47 cyc @1.2GHz), so the exp of all
    N^2 elems is a hard ~177us/core floor: ACT is the pacer.
  - Matmuls pay their LDWEIGHTS serially (no overlap measured), and f32r
    moving streams at ~half bf16 rate: baseline S-stage was 145us, PV-stage
    (P^T stationary, 128-col LDW churn) 127us -> PE-bound at ~272us.
  - Fixes: Q/K in fp16 (S matmul at bf16 speed, ~92us; fp16 rounding adds
    <0.3% exp error), PV in O^T orientation with V' = [V | 1] stationary
    (65-col weights, moving P^T: ~85us), so PE ~177us == ACT.
  - Finalize: device returns unnormalized O^T [65, N] (row 64 = softmax
    denominator from the ones column) straight to DRAM; host divides and
    transposes during unshard (O(N*d) postprocess, same bytes DMA'd).

Sharding: 32 (batch, head) pairs across 8 cores, 4 heads per core packed
as 2 pairs x 2 heads in 128 partitions (d=64 rows each).

Per-core schedule, per (pair, n-chunk of 1024), m-blocks of 128:
  PSUM: S_a, S_b [128,1024] f32 (4 banks), OT_a, OT_b [65,2,512] (4 banks).
  per mb: PV(mb-1) [4 MMs], S(mb) [4 MMs], exp_a(mb), exp_b(mb) [ACT].
  Emission order keeps the PE FIFO from blocking behind ACT-dependent work;
  ACT runs continuously, alternating heads (S single-buffered per head).
  Chunk end: DVE evacuates OT to SBUF fp32, DMA out; next chunk's PV
  start=True waits the evac (Tile serializes the PSUM bank reuse).
"""

import os
import sys

os.environ.setdefault("MYCRO_LOCAL_CACHE", "1")
sys.path.insert(0, "/opt/trn_rl_repo")

import numpy as np

import concourse.bacc as bacc
import concourse.mybir as mybir
import concourse.tile as tile
from concourse.bass_utils import run_bass_kernel_spmd

f32 = mybir.dt.float32
f16 = mybir.dt.float16
bf16 = mybir.dt.bfloat16

B, N, H, D = 2, 2048, 16, 64
HEADS_PER_CORE = 4
N_CORES = 8
NH = 1024          # n-chunk width
N_MB = N // 128    # 16 m-blocks of 128 rows
DV = D + 1         # V plus ones column
EXP = mybir.ActivationFunctionType.Exp


def emit_body(nc, qT, kT, vp, out, pools):
    qk_p, vt_p, spool, ppool, opool, finsb = pools

    # --- input DMAs ---
    qts, kts, vts = [], [], []
    for pair in range(2):
        qt = qk_p.tile([128, N], f16, name=f"qt_{pair}", tag=f"qt{pair}")
        kt = qk_p.tile([128, N], f16, name=f"kt_{pair}", tag=f"kt{pair}")
        nc.sync.dma_start(out=qt, in_=qT[pair])
        nc.sync.dma_start(out=kt, in_=kT[pair])
        qts.append(qt)
        kts.append(kt)
        for i in range(2):
            hh = 2 * pair + i
            vt = vt_p.tile([128, N_MB, DV], bf16, name=f"vt_{hh}", tag=f"vt{hh}")
            nc.sync.dma_start(
                out=vt, in_=vp[hh].rearrange("(mb p) d -> p mb d", p=128))
            vts.append(vt)

    for pair in range(2):
        qt, kt = qts[pair], kts[pair]
        for nh in range(2):
            sps = [
                spool.tile([128, NH], f32, name=f"s_{pair}_{nh}_{i}", tag=f"s{i}")
                for i in range(2)
            ]
            oaccs = [
                opool.tile([65, 2, 512], f32, name=f"o_{pair}_{nh}_{i}", tag=f"o{i}")
                for i in range(2)
            ]
            pts = {}
            for mb in range(N_MB + 1):
                # PV of previous m-block first so the PE FIFO never blocks
                # behind S matmuls that wait on ACT buffer frees.
                if mb > 0:
                    pmb = mb - 1
                    for i in range(2):
                        pt = pts[(pmb, i)]
                        for j in range(2):
                            nc.tensor.matmul(
                                out=oaccs[i][:, j, :],
                                lhsT=vts[2 * pair + i][:, pmb, :],
                                rhs=pt[:, j * 512:(j + 1) * 512],
                                start=(pmb == 0),
                                stop=(pmb == N_MB - 1))
                if mb < N_MB:
                    msl = slice(mb * 128, (mb + 1) * 128)
                    # S^T[m,n] = K Q^T, one head per 64-row group
                    for j in range(2):
                        jsl = slice(nh * NH + j * 512, nh * NH + (j + 1) * 512)
                        osl = slice(j * 512, (j + 1) * 512)
                        for i, plo in ((0, 0), (1, 64)):
                            nc.tensor.matmul(
                                out=sps[i][:, osl],
                                lhsT=kt[plo:plo + 64, msl],
                                rhs=qt[plo:plo + 64, jsl],
                                start=True, stop=True)
                    for i in range(2):
                        pt = ppool.tile([128, NH], bf16,
                                        name=f"pt_{pair}_{nh}_{mb}_{i}",
                                        tag=f"p{i}")
                        nc.scalar.activation(pt, sps[i], EXP)
                        pts[(mb, i)] = pt

            # finalize: evacuate unnormalized O^T (+denominator row) and DMA
            # out; host normalizes+transposes during unshard.
            for i in range(2):
                hh = 2 * pair + i
                osb = finsb.tile([65, NH], f32,
                                 name=f"osb_{pair}_{nh}_{i}", tag=f"os{i}")
                nc.vector.tensor_copy(
                    osb, oaccs[i].rearrange("p a b -> p (a b)"))
                nc.sync.dma_start(
                    out=out[hh][:, nh * NH:(nh + 1) * NH], in_=osb)


def build(repeat=1):
    nc = bacc.Bacc("TRN2", target_bir_lowering=False, debug=False)
    qT = nc.dram_tensor("qT", [2, 128, N], f16, kind="ExternalInput").ap()
    kT = nc.dram_tensor("kT", [2, 128, N], f16, kind="ExternalInput").ap()
    vp = nc.dram_tensor("vp", [HEADS_PER_CORE, N, DV], bf16,
                        kind="ExternalInput").ap()
    out = nc.dram_tensor("out", [HEADS_PER_CORE, DV, N], f32,
                         kind="ExternalOutput").ap()

    from contextlib import ExitStack
    with tile.TileContext(nc) as tc, ExitStack() as ctx:
        qk_p = ctx.enter_context(tc.tile_pool(name="qk", bufs=2))
        vt_p = ctx.enter_context(tc.tile_pool(name="vt", bufs=2))
        spool = ctx.enter_context(tc.tile_pool(name="spool", bufs=1, space="PSUM"))
        ppool = ctx.enter_context(tc.tile_pool(name="ppool", bufs=2))
        opool = ctx.enter_context(tc.tile_pool(name="opool", bufs=1, space="PSUM"))
        finsb = ctx.enter_context(tc.tile_pool(name="finsb", bufs=2))

        pools = (qk_p, vt_p, spool, ppool, opool, finsb)

        if repeat == 1:
            emit_body(nc, qT, kT, vp, out, pools)
        else:
            with tc.For_i(0, repeat, 1, hint_engines=(
                    mybir.EngineType.PE, mybir.EngineType.Activation,
                    mybir.EngineType.DVE, mybir.EngineType.SP)):
                emit_body(nc, qT, kT, vp, out, pools)

    nc.compile()
    return nc


_NC_CACHE = {}


def _get_nc(repeat=1):
    if repeat not in _NC_CACHE:
        _NC_CACHE[repeat] = build(repeat)
    return _NC_CACHE[repeat]


def _to_bf16(x):
    """Round fp32 -> bf16 (round-to-nearest-even), return uint16 view."""
    u = x.view(np.uint32)
    rounded = (u + 0x7FFF + ((u >> 16) & 1)) >> 16
    return rounded.astype(np.uint16)


def run_sharded(query, key, value, repeat=1, **spmd_kwargs):
    """query/key/value: [B, N, H, D] fp32 -> out [B, H, N, D] fp32."""
    import ml_dtypes
    nc = _get_nc(repeat)
    # [B, N, H, D] -> [B*H, D, N] fp16 for Q/K; [B*H, N, D+1] bf16 for V'
    qt = np.ascontiguousarray(
        np.transpose(query, (0, 2, 3, 1))).reshape(B * H, D, N).astype(np.float16)
    kt = np.ascontiguousarray(
        np.transpose(key, (0, 2, 3, 1))).reshape(B * H, D, N).astype(np.float16)
    vh = np.ascontiguousarray(np.transpose(value, (0, 2, 1, 3))).reshape(B * H, N, D)
    vp = np.empty((B * H, N, DV), dtype=np.uint16)
    vp[:, :, :D] = _to_bf16(vh)
    vp[:, :, D] = 0x3F80  # 1.0 in bf16
    vp = vp.view(ml_dtypes.bfloat16)
    in_maps = []
    for c in range(N_CORES):
        hs = slice(c * HEADS_PER_CORE, (c + 1) * HEADS_PER_CORE)
        in_maps.append({
            "qT": qt[hs].reshape(2, 128, N),
            "kT": kt[hs].reshape(2, 128, N),
            "vp": vp[hs],
        })
    res = run_bass_kernel_spmd(nc, in_maps, core_ids=list(range(N_CORES)),
                               **spmd_kwargs)
    # [8, 4, 65, N]: rows 0..63 = unnormalized O^T, row 64 = denominator
    outs = np.stack([res.results[c]["out"] for c in range(N_CORES)])
    num = outs[:, :, :D, :]          # [8, 4, 64, N]
    den = outs[:, :, D:D + 1, :]     # [8, 4, 1, N]
    o = (num / den).transpose(0, 1, 3, 2)  # [8, 4, N, 64]
    return np.ascontiguousarray(o.reshape(B, H, N, D).astype(np.float32))


def kernel(query, key, value):
    query = np.asarray(query, dtype=np.float32)
    key = np.asarray(key, dtype=np.float32)
    value = np.asarray(value, dtype=np.float32)
    return run_sharded(query, key, value)


if __name__ == "__main__":
    rng = np.random.default_rng(0)
    q = rng.standard_normal((B, N, H, D), dtype=np.float32)
    k = rng.standard_normal((B, N, H, D), dtype=np.float32)
    v = rng.standard_normal((B, N, H, D), dtype=np.float32)
    o = kernel(q, k, v)
    print("out shape:", o.shape, o.dtype)


# revision 5
# speedup vs baseline: 1.2029x; 1.2029x over previous
"""Exact attention (B=2, N=2048, H=16, D=64, fp32) on 8 Trainium2 NeuronCores.

v3 design, built from HW microbenchmarks (mb.py):
  - ACT exp measures ~1.23 ns/elem (1.47 cyc @1.2GHz), so the exp of all
    N^2 elems is a hard ~177us/core floor: ACT is the pacer.
  - Matmuls pay their LDWEIGHTS serially (no overlap measured), and f32r
    moving streams at ~half bf16 rate: baseline S-stage was 145us, PV-stage
    (P^T stationary, 128-col LDW churn) 127us -> PE-bound at ~272us.
  - Fixes: Q/K in fp16 (S matmul at bf16 speed, ~92us; fp16 rounding adds
    <0.3% exp error), PV in O^T orientation with V' = [V | 1] stationary
    (65-col weights, moving P^T: ~85us), so PE ~177us == ACT.
  - Finalize: device returns unnormalized O^T [65, N] (row 64 = softmax
    denominator from the ones column) straight to DRAM; host divides and
    transposes during unshard (O(N*d) postprocess, same bytes DMA'd).

Sharding: 32 (batch, head) pairs across 8 cores, 4 heads per core packed
as 2 pairs x 2 heads in 128 partitions (d=64 rows each).

Per-core schedule, per (pair, n-chunk of 1024), m-blocks of 128:
  PSUM: S_a, S_b [128,1024] f32 (4 banks), OT_a, OT_b [65,2,512] (4 banks).
  per mb: PV(mb-1) [4 MMs], S(mb) [4 MMs], exp_a(mb), exp_b(mb) [ACT].
  Emission order keeps the PE FIFO from blocking behind ACT-dependent work;
  ACT runs continuously, alternating heads (S single-buffered per head).
  Chunk end: DVE evacuates OT to SBUF fp32, DMA out; next chunk's PV
  start=True waits the evac (Tile serializes the PSUM bank reuse).
"""

import os
import sys

os.environ.setdefault("MYCRO_LOCAL_CACHE", "1")
sys.path.insert(0, "/opt/trn_rl_repo")

import numpy as np

import concourse.bacc as bacc
import concourse.mybir as mybir
import concourse.tile as tile
from concourse.bass_utils import run_bass_kernel_spmd

f32 = mybir.dt.float32
f16 = mybir.dt.float16
bf16 = mybir.dt.bfloat16

B, N, H, D = 2, 2048, 16, 64
HEADS_PER_CORE = 4
N_CORES = 8
NH = 1024          # n-chunk width
N_MB = N // 128    # 16 m-blocks of 128 rows
DV = D + 1         # V plus ones column
EXP = mybir.ActivationFunctionType.Exp


def emit_body(nc, qT, kT, vp, out, pools):
    qk_p, vt_p, spool, ppool, opool, finsb = pools

    # --- input DMAs ---
    qts, kts, vts = [], [], []
    for pair in range(2):
        qt = qk_p.tile([128, N], f16, name=f"qt_{pair}", tag=f"qt{pair}")
        kt = qk_p.tile([128, N], f16, name=f"kt_{pair}", tag=f"kt{pair}")
        nc.sync.dma_start(out=qt, in_=qT[pair])
        nc.sync.dma_start(out=kt, in_=kT[pair])
        qts.append(qt)
        kts.append(kt)
        for i in range(2):
            hh = 2 * pair + i
            vt = vt_p.tile([128, N_MB, DV], bf16, name=f"vt_{hh}", tag=f"vt{hh}")
            nc.sync.dma_start(
                out=vt, in_=vp[hh].rearrange("(mb p) d -> p mb d", p=128))
            vts.append(vt)

    # One head-chunk at a time: S double-buffered (4 banks) + one OT (2
    # banks) so S(mb+1) and PV(mb-1) both execute inside the exp(mb)
    # window. Heads alternate per chunk so oacc evac overlaps the next
    # chunk via the other tag.
    for nh in range(2):
        for pair in range(2):
            for i, plo in ((0, 0), (1, 64)):
                qt, kt = qts[pair], kts[pair]
                hh = 2 * pair + i
                oacc = opool.tile([65, 2, 512], f32,
                                  name=f"o_{pair}_{nh}_{i}", tag=f"o{i}")
                pts = {}
                for mb in range(N_MB + 1):
                    if mb < N_MB:
                        msl = slice(mb * 128, (mb + 1) * 128)
                        sp = spool.tile([128, NH], f32,
                                        name=f"s_{pair}_{nh}_{i}_{mb}", tag="s")
                        for j in range(2):
                            jsl = slice(nh * NH + j * 512, nh * NH + (j + 1) * 512)
                            nc.tensor.matmul(
                                out=sp[:, j * 512:(j + 1) * 512],
                                lhsT=kt[plo:plo + 64, msl],
                                rhs=qt[plo:plo + 64, jsl],
                                start=True, stop=True)
                    if mb > 0:
                        pmb = mb - 1
                        pt = pts.pop(pmb)
                        for j in range(2):
                            nc.tensor.matmul(
                                out=oacc[:, j, :],
                                lhsT=vts[hh][:, pmb, :],
                                rhs=pt[:, j * 512:(j + 1) * 512],
                                start=(pmb == 0),
                                stop=(pmb == N_MB - 1))
                    if mb < N_MB:
                        pt = ppool.tile([128, NH], bf16,
                                        name=f"pt_{pair}_{nh}_{i}_{mb}", tag="p")
                        nc.scalar.activation(pt, sp, EXP)
                        pts[mb] = pt

                # finalize: evacuate unnormalized O^T (+denominator row) and
                # DMA out; host normalizes+transposes during unshard.
                osb = finsb.tile([65, NH], f32,
                                 name=f"osb_{pair}_{nh}_{i}", tag=f"os{i}")
                nc.vector.tensor_copy(
                    osb, oacc.rearrange("p a b -> p (a b)"))
                nc.sync.dma_start(
                    out=out[hh][:, nh * NH:(nh + 1) * NH], in_=osb)


def build(repeat=1):
    nc = bacc.Bacc("TRN2", target_bir_lowering=False, debug=False)
    qT = nc.dram_tensor("qT", [2, 128, N], f16, kind="ExternalInput").ap()
    kT = nc.dram_tensor("kT", [2, 128, N], f16, kind="ExternalInput").ap()
    vp = nc.dram_tensor("vp", [HEADS_PER_CORE, N, DV], bf16,
                        kind="ExternalInput").ap()
    out = nc.dram_tensor("out", [HEADS_PER_CORE, DV, N], f32,
                         kind="ExternalOutput").ap()

    from contextlib import ExitStack
    with tile.TileContext(nc) as tc, ExitStack() as ctx:
        qk_p = ctx.enter_context(tc.tile_pool(name="qk", bufs=2))
        vt_p = ctx.enter_context(tc.tile_pool(name="vt", bufs=2))
        spool = ctx.enter_context(tc.tile_pool(name="spool", bufs=2, space="PSUM"))
        ppool = ctx.enter_context(tc.tile_pool(name="ppool", bufs=3))
        opool = ctx.enter_context(tc.tile_pool(name="opool", bufs=1, space="PSUM"))
        finsb = ctx.enter_context(tc.tile_pool(name="finsb", bufs=2))

        pools = (qk_p, vt_p, spool, ppool, opool, finsb)

        if repeat == 1:
            emit_body(nc, qT, kT, vp, out, pools)
        else:
            with tc.For_i(0, repeat, 1, hint_engines=(
                    mybir.EngineType.PE, mybir.EngineType.Activation,
                    mybir.EngineType.DVE, mybir.EngineType.SP)):
                emit_body(nc, qT, kT, vp, out, pools)

    nc.compile()
    return nc


_NC_CACHE = {}


def _get_nc(repeat=1):
    if repeat not in _NC_CACHE:
        _NC_CACHE[repeat] = build(repeat)
    return _NC_CACHE[repeat]


def _to_bf16(x):
    """Round fp32 -> bf16 (round-to-nearest-even), return uint16 view."""
    u = x.view(np.uint32)
    rounded = (u + 0x7FFF + ((u >> 16) & 1)) >> 16
    return rounded.astype(np.uint16)


def run_sharded(query, key, value, repeat=1, **spmd_kwargs):
    """query/key/value: [B, N, H, D] fp32 -> out [B, H, N, D] fp32."""
    import ml_dtypes
    nc = _get_nc(repeat)
    # [B, N, H, D] -> [B*H, D, N] fp16 for Q/K; [B*H, N, D+1] bf16 for V'
    qt = np.ascontiguousarray(
        np.transpose(query, (0, 2, 3, 1))).reshape(B * H, D, N).astype(np.float16)
    kt = np.ascontiguousarray(
        np.transpose(key, (0, 2, 3, 1))).reshape(B * H, D, N).astype(np.float16)
    vh = np.ascontiguousarray(np.transpose(value, (0, 2, 1, 3))).reshape(B * H, N, D)
    vp = np.empty((B * H, N, DV), dtype=np.uint16)
    vp[:, :, :D] = _to_bf16(vh)
    vp[:, :, D] = 0x3F80  # 1.0 in bf16
    vp = vp.view(ml_dtypes.bfloat16)
    in_maps = []
    for c in range(N_CORES):
        hs = slice(c * HEADS_PER_CORE, (c + 1) * HEADS_PER_CORE)
        in_maps.append({
            "qT": qt[hs].reshape(2, 128, N),
            "kT": kt[hs].reshape(2, 128, N),
            "vp": vp[hs],
        })
    res = run_bass_kernel_spmd(nc, in_maps, core_ids=list(range(N_CORES)),
                               **spmd_kwargs)
    # [8, 4, 65, N]: rows 0..63 = unnormalized O^T, row 64 = denominator
    outs = np.stack([res.results[c]["out"] for c in range(N_CORES)])
    num = outs[:, :, :D, :]          # [8, 4, 64, N]
    den = outs[:, :, D:D + 1, :]     # [8, 4, 1, N]
    o = (num / den).transpose(0, 1, 3, 2)  # [8, 4, N, 64]
    return np.ascontiguousarray(o.reshape(B, H, N, D).astype(np.float32))


def kernel(query, key, value):
    query = np.asarray(query, dtype=np.float32)
    key = np.asarray(key, dtype=np.float32)
    value = np.asarray(value, dtype=np.float32)
    return run_sharded(query, key, value)


if __name__ == "__main__":
    rng = np.random.default_rng(0)
    q = rng.standard_normal((B, N, H, D), dtype=np.float32)
    k = rng.standard_normal((B, N, H, D), dtype=np.float32)
    v = rng.standard_normal((B, N, H, D), dtype=np.float32)
    o = kernel(q, k, v)
    print("out shape:", o.shape, o.dtype)


# revision 8
# speedup vs baseline: 1.3311x; 1.1066x over previous
"""Exact attention (B=2, N=2048, H=16, D=64, fp32) on 8 Trainium2 NeuronCores.

v3 design, built from HW microbenchmarks (mb.py):
  - ACT exp measures ~1.23 ns/elem (1.47 cyc @1.2GHz), so the exp of all
    N^2 elems is a hard ~177us/core floor: ACT is the pacer.
  - Matmuls pay their LDWEIGHTS serially (no overlap measured), and f32r
    moving streams at ~half bf16 rate: baseline S-stage was 145us, PV-stage
    (P^T stationary, 128-col LDW churn) 127us -> PE-bound at ~272us.
  - Fixes: Q/K in fp16 (S matmul at bf16 speed, ~92us; fp16 rounding adds
    <0.3% exp error), PV in O^T orientation with V' = [V | 1] stationary
    (65-col weights, moving P^T: ~85us), so PE ~177us == ACT.
  - Finalize: device returns unnormalized O^T [65, N] (row 64 = softmax
    denominator from the ones column) straight to DRAM; host divides and
    transposes during unshard (O(N*d) postprocess, same bytes DMA'd).

Sharding: 32 (batch, head) pairs across 8 cores, 4 heads per core packed
as 2 pairs x 2 heads in 128 partitions (d=64 rows each).

Per-core schedule, per (pair, n-chunk of 1024), m-blocks of 128:
  PSUM: S_a, S_b [128,1024] f32 (4 banks), OT_a, OT_b [65,2,512] (4 banks).
  per mb: PV(mb-1) [4 MMs], S(mb) [4 MMs], exp_a(mb), exp_b(mb) [ACT].
  Emission order keeps the PE FIFO from blocking behind ACT-dependent work;
  ACT runs continuously, alternating heads (S single-buffered per head).
  Chunk end: DVE evacuates OT to SBUF fp32, DMA out; next chunk's PV
  start=True waits the evac (Tile serializes the PSUM bank reuse).
"""

import os
import sys

os.environ.setdefault("MYCRO_LOCAL_CACHE", "1")
sys.path.insert(0, "/opt/trn_rl_repo")

import numpy as np

import concourse.bacc as bacc
import concourse.mybir as mybir
import concourse.tile as tile
from concourse.bass_utils import run_bass_kernel_spmd

f32 = mybir.dt.float32
f16 = mybir.dt.float16
bf16 = mybir.dt.bfloat16

B, N, H, D = 2, 2048, 16, 64
HEADS_PER_CORE = 4
N_CORES = 8
NH = 1024          # n-chunk width
N_MB = N // 128    # 16 m-blocks of 128 rows
DV = D + 1         # V plus ones column
EXP = mybir.ActivationFunctionType.Exp


def emit_body(nc, qT, kT, vp, out, pools):
    qk_p, vt_p, spool, ppool, opool, finsb = pools

    # --- input DMAs ---
    qts, kts, vts = [], [], []
    for pair in range(2):
        qt = qk_p.tile([128, N], f16, name=f"qt_{pair}", tag=f"qt{pair}")
        kt = qk_p.tile([128, N], f16, name=f"kt_{pair}", tag=f"kt{pair}")
        nc.sync.dma_start(out=qt, in_=qT[pair])
        nc.sync.dma_start(out=kt, in_=kT[pair])
        qts.append(qt)
        kts.append(kt)
        for i in range(2):
            hh = 2 * pair + i
            vt = vt_p.tile([128, N_MB, DV], bf16, name=f"vt_{hh}", tag=f"vt{hh}")
            nc.sync.dma_start(
                out=vt, in_=vp[hh].rearrange("(mb p) d -> p mb d", p=128))
            vts.append(vt)

    # One head-chunk at a time: S double-buffered (4 banks) + one OT (2
    # banks) so S(mb+1) and PV(mb-1) both execute inside the exp(mb)
    # window. Heads alternate per chunk so oacc evac overlaps the next
    # chunk via the other tag.
    for nh in range(2):
        for pair in range(2):
            for i, plo in ((0, 0), (1, 64)):
                qt, kt = qts[pair], kts[pair]
                hh = 2 * pair + i
                oacc = opool.tile([65, 2, 512], f32,
                                  name=f"o_{pair}_{nh}_{i}", tag="o")
                pts = {}
                # PV lags S by 2 m-blocks: S(mb+1) completes mid-window so
                # its done-sem fires before ACT goes idle; PV fills the tail.
                LAG = 2
                for mb in range(N_MB + LAG):
                    if mb < N_MB:
                        msl = slice(mb * 128, (mb + 1) * 128)
                        sp = spool.tile([128, NH], f32,
                                        name=f"s_{pair}_{nh}_{i}_{mb}", tag="s")
                        for j in range(2):
                            jsl = slice(nh * NH + j * 512, nh * NH + (j + 1) * 512)
                            nc.tensor.matmul(
                                out=sp[:, j * 512:(j + 1) * 512],
                                lhsT=kt[plo:plo + 64, msl],
                                rhs=qt[plo:plo + 64, jsl],
                                start=True, stop=True)
                    if mb >= LAG:
                        pmb = mb - LAG
                        pt = pts.pop(pmb)
                        for j in range(2):
                            nc.tensor.matmul(
                                out=oacc[:, j, :],
                                lhsT=vts[hh][:, pmb, :],
                                rhs=pt[:, j * 512:(j + 1) * 512],
                                start=(pmb == 0),
                                stop=(pmb == N_MB - 1))
                    if mb < N_MB:
                        pt = ppool.tile([128, NH], bf16,
                                        name=f"pt_{pair}_{nh}_{i}_{mb}", tag="p")
                        nc.scalar.activation(pt, sp, EXP)
                        pts[mb] = pt

                # finalize: evacuate unnormalized O^T (+denominator row) and
                # DMA out; host normalizes+transposes during unshard.
                osb = finsb.tile([65, NH], f32,
                                 name=f"osb_{pair}_{nh}_{i}", tag=f"os{i}")
                nc.vector.tensor_copy(
                    osb, oacc.rearrange("p a b -> p (a b)"))
                nc.sync.dma_start(
                    out=out[hh][:, nh * NH:(nh + 1) * NH], in_=osb)


def build(repeat=1):
    nc = bacc.Bacc("TRN2", target_bir_lowering=False, debug=False)
    qT = nc.dram_tensor("qT", [2, 128, N], f16, kind="ExternalInput").ap()
    kT = nc.dram_tensor("kT", [2, 128, N], f16, kind="ExternalInput").ap()
    vp = nc.dram_tensor("vp", [HEADS_PER_CORE, N, DV], bf16,
                        kind="ExternalInput").ap()
    out = nc.dram_tensor("out", [HEADS_PER_CORE, DV, N], f32,
                         kind="ExternalOutput").ap()

    from contextlib import ExitStack
    with tile.TileContext(nc) as tc, ExitStack() as ctx:
        qk_p = ctx.enter_context(tc.tile_pool(name="qk", bufs=2))
        vt_p = ctx.enter_context(tc.tile_pool(name="vt", bufs=2))
        spool = ctx.enter_context(tc.tile_pool(name="spool", bufs=3, space="PSUM"))
        ppool = ctx.enter_context(tc.tile_pool(name="ppool", bufs=4))
        opool = ctx.enter_context(tc.tile_pool(name="opool", bufs=1, space="PSUM"))
        finsb = ctx.enter_context(tc.tile_pool(name="finsb", bufs=2))

        pools = (qk_p, vt_p, spool, ppool, opool, finsb)

        if repeat == 1:
            emit_body(nc, qT, kT, vp, out, pools)
        else:
            with tc.For_i(0, repeat, 1, hint_engines=(
                    mybir.EngineType.PE, mybir.EngineType.Activation,
                    mybir.EngineType.DVE, mybir.EngineType.SP)):
                emit_body(nc, qT, kT, vp, out, pools)

    nc.compile()
    return nc


_NC_CACHE = {}


def _get_nc(repeat=1):
    if repeat not in _NC_CACHE:
        _NC_CACHE[repeat] = build(repeat)
    return _NC_CACHE[repeat]


def _to_bf16(x):
    """Round fp32 -> bf16 (round-to-nearest-even), return uint16 view."""
    u = x.view(np.uint32)
    rounded = (u + 0x7FFF + ((u >> 16) & 1)) >> 16
    return rounded.astype(np.uint16)


def run_sharded(query, key, value, repeat=1, **spmd_kwargs):
    """query/key/value: [B, N, H, D] fp32 -> out [B, H, N, D] fp32."""
    import ml_dtypes
    nc = _get_nc(repeat)
    # [B, N, H, D] -> [B*H, D, N] fp16 for Q/K; [B*H, N, D+1] bf16 for V'
    qt = np.ascontiguousarray(
        np.transpose(query, (0, 2, 3, 1))).reshape(B * H, D, N).astype(np.float16)
    kt = np.ascontiguousarray(
        np.transpose(key, (0, 2, 3, 1))).reshape(B * H, D, N).astype(np.float16)
    vh = np.ascontiguousarray(np.transpose(value, (0, 2, 1, 3))).reshape(B * H, N, D)
    vp = np.empty((B * H, N, DV), dtype=np.uint16)
    vp[:, :, :D] = _to_bf16(vh)
    vp[:, :, D] = 0x3F80  # 1.0 in bf16
    vp = vp.view(ml_dtypes.bfloat16)
    in_maps = []
    for c in range(N_CORES):
        hs = slice(c * HEADS_PER_CORE, (c + 1) * HEADS_PER_CORE)
        in_maps.append({
            "qT": qt[hs].reshape(2, 128, N),
            "kT": kt[hs].reshape(2, 128, N),
            "vp": vp[hs],
        })
    res = run_bass_kernel_spmd(nc, in_maps, core_ids=list(range(N_CORES)),
                               **spmd_kwargs)
    # [8, 4, 65, N]: rows 0..63 = unnormalized O^T, row 64 = denominator
    outs = np.stack([res.results[c]["out"] for c in range(N_CORES)])
    num = outs[:, :, :D, :]          # [8, 4, 64, N]
    den = outs[:, :, D:D + 1, :]     # [8, 4, 1, N]
    o = (num / den).transpose(0, 1, 3, 2)  # [8, 4, N, 64]
    return np.ascontiguousarray(o.reshape(B, H, N, D).astype(np.float32))


def kernel(query, key, value):
    query = np.asarray(query, dtype=np.float32)
    key = np.asarray(key, dtype=np.float32)
    value = np.asarray(value, dtype=np.float32)
    return run_sharded(query, key, value)


if __name__ == "__main__":
    rng = np.random.default_rng(0)
    q = rng.standard_normal((B, N, H, D), dtype=np.float32)
    k = rng.standard_normal((B, N, H, D), dtype=np.float32)
    v = rng.standard_normal((B, N, H, D), dtype=np.float32)
    o = kernel(q, k, v)
    print("out shape:", o.shape, o.dtype)


# revision 9
# speedup vs baseline: 1.4313x; 1.0752x over previous
"""Exact attention (B=2, N=2048, H=16, D=64, fp32) on 8 Trainium2 NeuronCores.

v3 design, built from HW microbenchmarks (mb.py):
  - ACT exp measures ~1.23 ns/elem (1.47 cyc @1.2GHz), so the exp of all
    N^2 elems is a hard ~177us/core floor: ACT is the pacer.
  - Matmuls pay their LDWEIGHTS serially (no overlap measured), and f32r
    moving streams at ~half bf16 rate: baseline S-stage was 145us, PV-stage
    (P^T stationary, 128-col LDW churn) 127us -> PE-bound at ~272us.
  - Fixes: Q/K in fp16 (S matmul at bf16 speed, ~92us; fp16 rounding adds
    <0.3% exp error), PV in O^T orientation with V' = [V | 1] stationary
    (65-col weights, moving P^T: ~85us), so PE ~177us == ACT.
  - Finalize: device returns unnormalized O^T [65, N] (row 64 = softmax
    denominator from the ones column) straight to DRAM; host divides and
    transposes during unshard (O(N*d) postprocess, same bytes DMA'd).

Sharding: 32 (batch, head) pairs across 8 cores, 4 heads per core packed
as 2 pairs x 2 heads in 128 partitions (d=64 rows each).

Per-core schedule, per (pair, n-chunk of 1024), m-blocks of 128:
  PSUM: S_a, S_b [128,1024] f32 (4 banks), OT_a, OT_b [65,2,512] (4 banks).
  per mb: PV(mb-1) [4 MMs], S(mb) [4 MMs], exp_a(mb), exp_b(mb) [ACT].
  Emission order keeps the PE FIFO from blocking behind ACT-dependent work;
  ACT runs continuously, alternating heads (S single-buffered per head).
  Chunk end: DVE evacuates OT to SBUF fp32, DMA out; next chunk's PV
  start=True waits the evac (Tile serializes the PSUM bank reuse).
"""

import os
import sys

os.environ.setdefault("MYCRO_LOCAL_CACHE", "1")
sys.path.insert(0, "/opt/trn_rl_repo")

import numpy as np

import concourse.bacc as bacc
import concourse.mybir as mybir
import concourse.tile as tile
from concourse.bass_utils import run_bass_kernel_spmd

f32 = mybir.dt.float32
f16 = mybir.dt.float16
bf16 = mybir.dt.bfloat16

B, N, H, D = 2, 2048, 16, 64
HEADS_PER_CORE = 4
N_CORES = 8
NH = 1024          # n-chunk width
N_MB = N // 128    # 16 m-blocks of 128 rows
DV = D + 1         # V plus ones column
EXP = mybir.ActivationFunctionType.Exp


def emit_body(nc, qT, kT, vp, out, pools):
    qk_p, vt_p, spool, ppool, opool, finsb = pools

    # --- input DMAs ---
    qts, kts, vts = [], [], []
    for pair in range(2):
        qt = qk_p.tile([128, N], f16, name=f"qt_{pair}", tag=f"qt{pair}")
        kt = qk_p.tile([128, N], f16, name=f"kt_{pair}", tag=f"kt{pair}")
        nc.sync.dma_start(out=qt, in_=qT[pair])
        nc.sync.dma_start(out=kt, in_=kT[pair])
        qts.append(qt)
        kts.append(kt)
        for i in range(2):
            hh = 2 * pair + i
            vt = vt_p.tile([128, N_MB, DV], bf16, name=f"vt_{hh}", tag=f"vt{hh}")
            nc.sync.dma_start(
                out=vt, in_=vp[hh].rearrange("(mb p) d -> p mb d", p=128))
            vts.append(vt)

    # One head-chunk at a time: S double-buffered (4 banks) + one OT (2
    # banks) so S(mb+1) and PV(mb-1) both execute inside the exp(mb)
    # window. Heads alternate per chunk so oacc evac overlaps the next
    # chunk via the other tag.
    for nh in range(2):
        for pair in range(2):
            for i, plo in ((0, 0), (1, 64)):
                qt, kt = qts[pair], kts[pair]
                hh = 2 * pair + i
                oacc = opool.tile([65, 2, 512], f32,
                                  name=f"o_{pair}_{nh}_{i}", tag="o")
                pts = {}
                # PV lags S by 2 m-blocks: S(mb+1) completes mid-window so
                # its done-sem fires before ACT goes idle; PV fills the tail.
                LAG = 3
                for mb in range(N_MB + LAG):
                    if mb < N_MB:
                        msl = slice(mb * 128, (mb + 1) * 128)
                        sp = spool.tile([128, NH], f32,
                                        name=f"s_{pair}_{nh}_{i}_{mb}", tag="s")
                        for j in range(2):
                            jsl = slice(nh * NH + j * 512, nh * NH + (j + 1) * 512)
                            nc.tensor.matmul(
                                out=sp[:, j * 512:(j + 1) * 512],
                                lhsT=kt[plo:plo + 64, msl],
                                rhs=qt[plo:plo + 64, jsl],
                                start=True, stop=True)
                    if mb >= LAG:
                        pmb = mb - LAG
                        pt = pts.pop(pmb)
                        for j in range(2):
                            nc.tensor.matmul(
                                out=oacc[:, j, :],
                                lhsT=vts[hh][:, pmb, :],
                                rhs=pt[:, j * 512:(j + 1) * 512],
                                start=(pmb == 0),
                                stop=(pmb == N_MB - 1))
                    if mb < N_MB:
                        pt = ppool.tile([128, NH], bf16,
                                        name=f"pt_{pair}_{nh}_{i}_{mb}", tag="p")
                        nc.scalar.activation(pt, sp, EXP)
                        pts[mb] = pt

                # finalize: evacuate unnormalized O^T (+denominator row) and
                # DMA out; host normalizes+transposes during unshard.
                osb = finsb.tile([65, NH], f32,
                                 name=f"osb_{pair}_{nh}_{i}", tag=f"os{i}")
                nc.vector.tensor_copy(
                    osb, oacc.rearrange("p a b -> p (a b)"))
                nc.sync.dma_start(
                    out=out[hh][:, nh * NH:(nh + 1) * NH], in_=osb)


def build(repeat=1):
    nc = bacc.Bacc("TRN2", target_bir_lowering=False, debug=False)
    qT = nc.dram_tensor("qT", [2, 128, N], f16, kind="ExternalInput").ap()
    kT = nc.dram_tensor("kT", [2, 128, N], f16, kind="ExternalInput").ap()
    vp = nc.dram_tensor("vp", [HEADS_PER_CORE, N, DV], bf16,
                        kind="ExternalInput").ap()
    out = nc.dram_tensor("out", [HEADS_PER_CORE, DV, N], f32,
                         kind="ExternalOutput").ap()

    from contextlib import ExitStack
    with tile.TileContext(nc) as tc, ExitStack() as ctx:
        qk_p = ctx.enter_context(tc.tile_pool(name="qk", bufs=2))
        vt_p = ctx.enter_context(tc.tile_pool(name="vt", bufs=2))
        spool = ctx.enter_context(tc.tile_pool(name="spool", bufs=3, space="PSUM"))
        ppool = ctx.enter_context(tc.tile_pool(name="ppool", bufs=6))
        opool = ctx.enter_context(tc.tile_pool(name="opool", bufs=1, space="PSUM"))
        finsb = ctx.enter_context(tc.tile_pool(name="finsb", bufs=2))

        pools = (qk_p, vt_p, spool, ppool, opool, finsb)

        if repeat == 1:
            emit_body(nc, qT, kT, vp, out, pools)
        else:
            with tc.For_i(0, repeat, 1, hint_engines=(
                    mybir.EngineType.PE, mybir.EngineType.Activation,
                    mybir.EngineType.DVE, mybir.EngineType.SP)):
                emit_body(nc, qT, kT, vp, out, pools)

    nc.compile()
    return nc


_NC_CACHE = {}


def _get_nc(repeat=1):
    if repeat not in _NC_CACHE:
        _NC_CACHE[repeat] = build(repeat)
    return _NC_CACHE[repeat]


def _to_bf16(x):
    """Round fp32 -> bf16 (round-to-nearest-even), return uint16 view."""
    u = x.view(np.uint32)
    rounded = (u + 0x7FFF + ((u >> 16) & 1)) >> 16
    return rounded.astype(np.uint16)


def run_sharded(query, key, value, repeat=1, **spmd_kwargs):
    """query/key/value: [B, N, H, D] fp32 -> out [B, H, N, D] fp32."""
    import ml_dtypes
    nc = _get_nc(repeat)
    # [B, N, H, D] -> [B*H, D, N] fp16 for Q/K; [B*H, N, D+1] bf16 for V'
    qt = np.ascontiguousarray(
        np.transpose(query, (0, 2, 3, 1))).reshape(B * H, D, N).astype(np.float16)
    kt = np.ascontiguousarray(
        np.transpose(key, (0, 2, 3, 1))).reshape(B * H, D, N).astype(np.float16)
    vh = np.ascontiguousarray(np.transpose(value, (0, 2, 1, 3))).reshape(B * H, N, D)
    vp = np.empty((B * H, N, DV), dtype=np.uint16)
    vp[:, :, :D] = _to_bf16(vh)
    vp[:, :, D] = 0x3F80  # 1.0 in bf16
    vp = vp.view(ml_dtypes.bfloat16)
    in_maps = []
    for c in range(N_CORES):
        hs = slice(c * HEADS_PER_CORE, (c + 1) * HEADS_PER_CORE)
        in_maps.append({
            "qT": qt[hs].reshape(2, 128, N),
            "kT": kt[hs].reshape(2, 128, N),
            "vp": vp[hs],
        })
    res = run_bass_kernel_spmd(nc, in_maps, core_ids=list(range(N_CORES)),
                               **spmd_kwargs)
    # [8, 4, 65, N]: rows 0..63 = unnormalized O^T, row 64 = denominator
    outs = np.stack([res.results[c]["out"] for c in range(N_CORES)])
    num = outs[:, :, :D, :]          # [8, 4, 64, N]
    den = outs[:, :, D:D + 1, :]     # [8, 4, 1, N]
    o = (num / den).transpose(0, 1, 3, 2)  # [8, 4, N, 64]
    return np.ascontiguousarray(o.reshape(B, H, N, D).astype(np.float32))


def kernel(query, key, value):
    query = np.asarray(query, dtype=np.float32)
    key = np.asarray(key, dtype=np.float32)
    value = np.asarray(value, dtype=np.float32)
    return run_sharded(query, key, value)


if __name__ == "__main__":
    rng = np.random.default_rng(0)
    q = rng.standard_normal((B, N, H, D), dtype=np.float32)
    k = rng.standard_normal((B, N, H, D), dtype=np.float32)
    v = rng.standard_normal((B, N, H, D), dtype=np.float32)
    o = kernel(q, k, v)
    print("out shape:", o.shape, o.dtype)
